# revision 1
# baseline (speedup 1.0000x reference)
"""Squared-Euclidean-distance kernel for Trainium2 (8 NeuronCores, SPMD).

Computes out[b,n,u] = sum_d (x[b,n,d] - w[d,u])^2 for
x [8, 4096, 128] f32, w [128, 1024] f32 -> out [8, 4096, 1024] f32,
via the algebraic identity |x|^2 + |w|^2 - 2 x.w.

Distribution: data-parallel over the batch dim — core c handles x[c]
([4096, 128] rows), w replicated. No cross-core communication.

Per-core device kernel:
  - host precomputes xt = x[c].T (d on partitions, fp16), wneg2 = -2w
    (fp16), x2 (per-point squared norms f32, [128, 32] column-per-n-tile)
    and an aux row [ones | |w_u|^2] used to broadcast w2 on-device.
  - w2p = ones^T @ w2 (K=1 matmul) -> ScalarE copy to SBUF, built while
    inputs stream in.
  - 32 n-tiles of 128 points: PSUM[128,1024] = xt_tile.T @ wneg2 (2
    matmuls of free-dim 512), then ScalarE adds x2 (per-partition bias)
    while copying PSUM->SBUF, VectorE adds w2p, DMA to HBM.
The GEMM runs in fp16 (full PE rate, 2-byte operands); the rank-1
|x|^2/|w|^2 terms are f32, keeping total relative error ~1e-4.
"""

import sys
import types

try:
    import concourse.bass as bass  # noqa: F401
except ImportError:  # fresh interpreter without the repo on sys.path
    sys.path.insert(0, "/opt/trn_rl_repo")

import numpy as np

import concourse.bass as bass
import concourse.bacc as bacc
import concourse.tile as tile
import concourse.mybir as mybir
import concourse.bass_utils as bass_utils
from concourse.bass_utils import run_bass_kernel_spmd

B, N, D, U = 8, 4096, 128, 1024
N_CORES = 8
P = 128
N_TILES = N // P          # 32 n-tiles per core
U_HALF = 512              # PSUM bank = 512 f32
XT_CHUNK = 512            # xt loaded as 8 chunks of [128, 512]

# GEMM operand dtype: float32 (exact, 4 cyc/col), float32r (fp32 bits,
# full-rate 1 cyc/col, ~1e-4 rel accuracy), float16 (full rate, 2-byte
# inputs, ~3e-4) or bfloat16 (~2e-3).
# The |x|^2 / |w|^2 rank-1 terms always stay f32 (added outside the PE).
GEMM_DT = mybir.dt.float16
GEMM_NP = np.float16


def _install_ntff_hook():
    """Wire the NTFF profile hook the agent image leaves unconnected."""
    if "antenv.axon_hooks" in sys.modules:
        return
    try:
        from trn_agent_boot.trn_boot import _ntff_profile_via_ctypes
        hook = _ntff_profile_via_ctypes("/opt/axon/libaxon_pjrt.so")
    except Exception:
        hook = None
    mod = types.ModuleType("antenv.axon_hooks")
    mod.get_axon_ntff_profile_hook = lambda: hook
    mod.set_axon_ntff_profile_hook = lambda h: None
    sys.modules["antenv.axon_hooks"] = mod
    bass_utils.upload_artifacts = lambda tmpdir: f"local://{tmpdir}"


def build_bass(gemm_dt=None):
    """Build + compile the per-core Bass program (SPMD, same on all cores)."""
    gemm_dt = gemm_dt or GEMM_DT
    nc = bacc.Bacc("TRN2", target_bir_lowering=False, debug=False,
                   enable_asserts=True, num_devices=N_CORES)

    xt_ap = nc.dram_tensor("xt", [P, N], gemm_dt, kind="ExternalInput").ap()
    wneg2_ap = nc.dram_tensor("wneg2", [P, U], gemm_dt, kind="ExternalInput").ap()
    x2_ap = nc.dram_tensor("x2", [P, N_TILES], mybir.dt.float32,
                           kind="ExternalInput").ap()
    # aux row: [ones(128) | w2(1024)] in float32r, for the K=1 broadcast mm
    aux_ap = nc.dram_tensor("aux", [1, P + U], mybir.dt.float32r,
                            kind="ExternalInput").ap()
    out_ap = nc.dram_tensor("out", [N, U], mybir.dt.float32,
                            kind="ExternalOutput").ap()

    with tile.TileContext(nc) as tc:
        with (
            tc.tile_pool(name="singles", bufs=1) as singles,
            tc.tile_pool(name="xchunks", bufs=N // XT_CHUNK) as xchunks,
            tc.tile_pool(name="psum", bufs=4, space="PSUM") as psum_pool,
            tc.tile_pool(name="outs", bufs=8) as out_pool,
        ):
            # Load order matters: the first n-tile's pipeline needs wneg2 +
            # xt chunk 0 + x2 + w2, so issue those first; the rest
            # overlaps with compute.
            # aux goes first: it is tiny and the w2p broadcast build (PE
            # ones-matmul + ScalarE copy) runs while the real inputs load.
            aux_sb = singles.tile([1, P + U], mybir.dt.float32r, tag="aux")
            nc.sync.dma_start(aux_sb[:], aux_ap[:])
            # wneg2 in two half tiles so tile 0's first matmul only waits
            # for the first 0.125 MiB.
            wneg2_h = []
            for h in range(U // U_HALF):
                wtile = singles.tile([P, U_HALF], gemm_dt, tag=f"wneg2{h}",
                                     name=f"wneg2{h}")
                wneg2_h.append(wtile)
            nc.sync.dma_start(wneg2_h[0][:], wneg2_ap[:, 0:U_HALF])
            xt_sbs = []
            for ci in range(N // XT_CHUNK):
                t = xchunks.tile([P, XT_CHUNK], gemm_dt, tag=f"xt{ci}")
                xt_sbs.append(t)
            nc.sync.dma_start(xt_sbs[0][:], xt_ap[:, 0:XT_CHUNK])
            x2_sb = singles.tile([P, N_TILES], mybir.dt.float32, tag="x2")
            nc.sync.dma_start(x2_sb[:], x2_ap[:])
            nc.sync.dma_start(wneg2_h[1][:], wneg2_ap[:, U_HALF:U])
            # |w_u|^2 broadcast to all partitions: K=1 ones-matmul through a
            # transient PSUM slot + ScalarE copies (replaces a 512 KiB DMA).
            w2p_ps = psum_pool.tile([P, U], mybir.dt.float32, tag="acc")
            for h in range(U // U_HALF):
                nc.tensor.matmul(
                    w2p_ps[:, h * U_HALF:(h + 1) * U_HALF],
                    aux_sb[:, 0:P],
                    aux_sb[:, P + h * U_HALF:P + (h + 1) * U_HALF],
                    start=True, stop=True,
                )
            w2p_sb = singles.tile([P, U], mybir.dt.float32, tag="w2p")
            for h in range(U // U_HALF):
                sl = slice(h * U_HALF, (h + 1) * U_HALF)
                nc.scalar.copy(w2p_sb[:, sl], w2p_ps[:, sl])
            for ci in range(1, N // XT_CHUNK):
                nc.sync.dma_start(xt_sbs[ci][:],
                                  xt_ap[:, ci * XT_CHUNK:(ci + 1) * XT_CHUNK])

            tiles_per_chunk = XT_CHUNK // P
            for j in range(N_TILES):
                chunk = xt_sbs[j // tiles_per_chunk]
                col0 = (j % tiles_per_chunk) * P
                lhsT = chunk[:, col0:col0 + P]

                acc = psum_pool.tile([P, U], mybir.dt.float32, tag="acc")
                for h in range(U // U_HALF):
                    nc.tensor.matmul(
                        acc[:, h * U_HALF:(h + 1) * U_HALF],
                        lhsT,
                        wneg2_h[h][:],
                        start=True, stop=True,
                    )

                o = out_pool.tile([P, U], mybir.dt.float32, tag="o")
                # Epilogue: o = acc + x2[:, j] (ScalarE bias-add), then
                # o += w2p (VectorE), then DMA out. The first tiles are
                # processed per u-half so the output stream starts sooner.
                n_pieces = 2 if j < 2 else 1
                pw = U // n_pieces
                for pc in range(n_pieces):
                    sl = slice(pc * pw, (pc + 1) * pw)
                    nc.scalar.activation(
                        out=o[:, sl], in_=acc[:, sl],
                        func=mybir.ActivationFunctionType.Identity,
                        bias=x2_sb[:, j:j + 1], scale=1.0,
                    )
                    nc.vector.tensor_add(o[:, sl], o[:, sl], w2p_sb[:, sl])
                    nc.sync.dma_start(out_ap[j * P:(j + 1) * P, sl], o[:, sl])

    nc.compile()
    return nc


_CACHED_NC = None


def _get_nc():
    global _CACHED_NC
    if _CACHED_NC is None:
        _CACHED_NC = build_bass()
    return _CACHED_NC


def make_in_maps(x, w, gemm_np=None):
    """Host-side shard + precompute: per-core input dict list."""
    gemm_np = gemm_np or GEMM_NP
    x = np.asarray(x, dtype=np.float32)
    w = np.asarray(w, dtype=np.float32)
    wneg2 = (-2.0 * w).astype(gemm_np)
    w2 = (w.astype(np.float64) ** 2).sum(axis=0).astype(np.float32)
    aux = np.concatenate([np.ones(P, np.float32), w2]).reshape(1, P + U)
    in_maps = []
    for c in range(N_CORES):
        xs = x[c]                                    # [4096, 128]
        xt = np.ascontiguousarray(xs.T).astype(gemm_np)       # [128, 4096]
        x2 = (xs ** 2).sum(axis=1, dtype=np.float32)          # [4096]
        x2cols = np.ascontiguousarray(x2.reshape(N_TILES, P).T)  # [128, 32]
        in_maps.append({"xt": xt, "wneg2": wneg2, "x2": x2cols, "aux": aux})
    return in_maps


def run(x, w, trace=False):
    _install_ntff_hook()
    nc = _get_nc()
    in_maps = make_in_maps(x, w)
    last_err = None
    for _attempt in range(3):
        try:
            res = run_bass_kernel_spmd(nc, in_maps,
                                       core_ids=list(range(N_CORES)),
                                       trace=trace)
            break
        except Exception as e:  # transient device/tunnel hiccups
            last_err = e
    else:
        raise last_err
    out = np.stack([res.results[c]["out"] for c in range(N_CORES)], axis=0)
    return out, res


def kernel(x, w):
    out, _ = run(x, w, trace=False)
    return out



# revision 2
# speedup vs baseline: 1.0862x; 1.0862x over previous
"""Squared-Euclidean-distance kernel for Trainium2 (8 NeuronCores, SPMD).

Computes out[b,n,u] = sum_d (x[b,n,d] - w[d,u])^2 for
x [8, 4096, 128] f32, w [128, 1024] f32 -> out [8, 4096, 1024] f32,
via the algebraic identity |x|^2 + |w|^2 - 2 x.w.

Distribution: data-parallel over the batch dim — core c handles x[c]
([4096, 128] rows), w replicated. No cross-core communication.

Per-core device kernel (DMA-bound; ~360 GB/s/core across 16 queues):
  - GEMM in fp16 (full PE rate): PSUM[128,1024] = xt_tile.T @ (-2w).
  - Output written to HBM as fp16 (halves the dominant output traffic;
    elementwise error ~5e-4 of scale) and widened to f32 on the host.
  - Epilogue split across engines so no engine exceeds the DMA budget:
    ScalarE (Act) converts cols [0:640) with the per-point |x|^2 bias,
    VectorE (DVE) does cols [640:1024) in one scalar_tensor_tensor
    ((acc + x2) + w2) plus a 4x-mode fp16 (+w2) fixup on Act's slice.
  - Output DMAs alternate between the SP and Act hardware DGE queues
    (descriptor dispatch is ~0.7 us/instruction on one sequencer).
"""

import sys
import types

try:
    import concourse.bass as bass  # noqa: F401
except ImportError:  # fresh interpreter without the repo on sys.path
    sys.path.insert(0, "/opt/trn_rl_repo")

import numpy as np

import concourse.bass as bass
import concourse.bacc as bacc
import concourse.tile as tile
import concourse.mybir as mybir
import concourse.bass_utils as bass_utils
from concourse.bass_utils import run_bass_kernel_spmd

B, N, D, U = 8, 4096, 128, 1024
N_CORES = 8
P = 128
N_TILES = N // P          # 32 n-tiles per core
U_HALF = 512              # PSUM bank = 512 f32
XT_CHUNK = 512            # xt loaded as 8 chunks of [128, 512]
ACT_U = 640               # epilogue cols done by ScalarE (rest on VectorE)

GEMM_DT = mybir.dt.float16
GEMM_NP = np.float16
OUT_DT = mybir.dt.float16
OUT_NP = np.float16


def _install_ntff_hook():
    """Wire the NTFF profile hook the agent image leaves unconnected."""
    if "antenv.axon_hooks" in sys.modules:
        return
    try:
        from trn_agent_boot.trn_boot import _ntff_profile_via_ctypes
        hook = _ntff_profile_via_ctypes("/opt/axon/libaxon_pjrt.so")
    except Exception:
        hook = None
    mod = types.ModuleType("antenv.axon_hooks")
    mod.get_axon_ntff_profile_hook = lambda: hook
    mod.set_axon_ntff_profile_hook = lambda h: None
    sys.modules["antenv.axon_hooks"] = mod
    bass_utils.upload_artifacts = lambda tmpdir: f"local://{tmpdir}"


def build_bass():
    """Build + compile the per-core Bass program (SPMD, same on all cores)."""
    nc = bacc.Bacc("TRN2", target_bir_lowering=False, debug=False,
                   enable_asserts=True, num_devices=N_CORES)

    xt_ap = nc.dram_tensor("xt", [P, N], GEMM_DT, kind="ExternalInput").ap()
    wneg2_ap = nc.dram_tensor("wneg2", [P, U], GEMM_DT, kind="ExternalInput").ap()
    x2_ap = nc.dram_tensor("x2", [P, N_TILES], mybir.dt.float32,
                           kind="ExternalInput").ap()
    # aux row: [ones(128) | w2(1024)] in float32r, for the K=1 broadcast mm
    aux_ap = nc.dram_tensor("aux", [1, P + U], mybir.dt.float32r,
                            kind="ExternalInput").ap()
    out_ap = nc.dram_tensor("out", [N, U], OUT_DT,
                            kind="ExternalOutput").ap()

    add = mybir.AluOpType.add

    with tile.TileContext(nc) as tc:
        with (
            tc.tile_pool(name="singles", bufs=1) as singles,
            tc.tile_pool(name="xchunks", bufs=N // XT_CHUNK) as xchunks,
            tc.tile_pool(name="psum", bufs=4, space="PSUM") as psum_pool,
            tc.tile_pool(name="outs", bufs=8) as out_pool,
        ):
            # aux first: tiny, and the w2 broadcast build (PE ones-matmul +
            # ScalarE copy) runs while the real inputs stream in.
            aux_sb = singles.tile([1, P + U], mybir.dt.float32r, tag="aux")
            nc.sync.dma_start(aux_sb[:], aux_ap[:])
            wneg2_h = []
            for h in range(U // U_HALF):
                wtile = singles.tile([P, U_HALF], GEMM_DT, tag=f"wneg2{h}",
                                     name=f"wneg2{h}")
                wneg2_h.append(wtile)
            nc.sync.dma_start(wneg2_h[0][:], wneg2_ap[:, 0:U_HALF])
            xt_sbs = []
            for ci in range(N // XT_CHUNK):
                t = xchunks.tile([P, XT_CHUNK], GEMM_DT, tag=f"xt{ci}")
                xt_sbs.append(t)
            nc.sync.dma_start(xt_sbs[0][:], xt_ap[:, 0:XT_CHUNK])
            x2_sb = singles.tile([P, N_TILES], mybir.dt.float32, tag="x2")
            nc.scalar.dma_start(x2_sb[:], x2_ap[:])
            nc.scalar.dma_start(wneg2_h[1][:], wneg2_ap[:, U_HALF:U])
            # |w_u|^2 broadcast to all partitions: K=1 ones-matmul through a
            # transient PSUM slot, copied out in fp16 for the 4x DVE fixup.
            w2p_ps = psum_pool.tile([P, U], mybir.dt.float32, tag="acc")
            for h in range(U // U_HALF):
                nc.tensor.matmul(
                    w2p_ps[:, h * U_HALF:(h + 1) * U_HALF],
                    aux_sb[:, 0:P],
                    aux_sb[:, P + h * U_HALF:P + (h + 1) * U_HALF],
                    start=True, stop=True,
                )
            w2p_sb = singles.tile([P, U], OUT_DT, tag="w2p")
            for h in range(U // U_HALF):
                sl = slice(h * U_HALF, (h + 1) * U_HALF)
                nc.scalar.copy(w2p_sb[:, sl], w2p_ps[:, sl])
            for ci in range(1, N // XT_CHUNK):
                eng = nc.sync if ci % 2 == 0 else nc.scalar
                eng.dma_start(xt_sbs[ci][:],
                              xt_ap[:, ci * XT_CHUNK:(ci + 1) * XT_CHUNK])

            tiles_per_chunk = XT_CHUNK // P
            for j in range(N_TILES):
                chunk = xt_sbs[j // tiles_per_chunk]
                col0 = (j % tiles_per_chunk) * P
                lhsT = chunk[:, col0:col0 + P]

                acc = psum_pool.tile([P, U], mybir.dt.float32, tag="acc")
                for h in range(U // U_HALF):
                    nc.tensor.matmul(
                        acc[:, h * U_HALF:(h + 1) * U_HALF],
                        lhsT,
                        wneg2_h[h][:],
                        start=True, stop=True,
                    )

                o = out_pool.tile([P, U], OUT_DT, tag="o")
                x2col = x2_sb[:, j:j + 1]
                # ScalarE: cols [0:ACT_U) = acc + x2 (bias), f32->fp16.
                nc.scalar.activation(
                    out=o[:, 0:ACT_U], in_=acc[:, 0:ACT_U],
                    func=mybir.ActivationFunctionType.Identity,
                    bias=x2col, scale=1.0,
                )
                # VectorE: cols [ACT_U:U) = (acc + x2) + w2 in one op.
                nc.vector.scalar_tensor_tensor(
                    out=o[:, ACT_U:U], in0=acc[:, ACT_U:U], scalar=x2col,
                    in1=w2p_sb[:, ACT_U:U], op0=add, op1=add,
                )
                # VectorE 4x-mode fp16 fixup: += w2 on the ScalarE slice.
                nc.vector.scalar_tensor_tensor(
                    out=o[:, 0:ACT_U], in0=o[:, 0:ACT_U], scalar=0.0,
                    in1=w2p_sb[:, 0:ACT_U], op0=add, op1=add,
                )
                eng = nc.sync if j % 2 == 0 else nc.scalar
                eng.dma_start(out_ap[j * P:(j + 1) * P, :], o[:])

    nc.compile()
    return nc


_CACHED_NC = None


def _get_nc():
    global _CACHED_NC
    if _CACHED_NC is None:
        _CACHED_NC = build_bass()
    return _CACHED_NC


def make_in_maps(x, w):
    """Host-side shard + precompute: per-core input dict list."""
    x = np.asarray(x, dtype=np.float32)
    w = np.asarray(w, dtype=np.float32)
    wneg2 = (-2.0 * w).astype(GEMM_NP)
    w2 = (w.astype(np.float64) ** 2).sum(axis=0).astype(np.float32)
    aux = np.concatenate([np.ones(P, np.float32), w2]).reshape(1, P + U)
    in_maps = []
    for c in range(N_CORES):
        xs = x[c]                                    # [4096, 128]
        xt = np.ascontiguousarray(xs.T).astype(GEMM_NP)       # [128, 4096]
        x2 = (xs ** 2).sum(axis=1, dtype=np.float32)          # [4096]
        x2cols = np.ascontiguousarray(x2.reshape(N_TILES, P).T)  # [128, 32]
        in_maps.append({"xt": xt, "wneg2": wneg2, "x2": x2cols, "aux": aux})
    return in_maps


def run(x, w, trace=False):
    _install_ntff_hook()
    nc = _get_nc()
    in_maps = make_in_maps(x, w)
    last_err = None
    for _attempt in range(3):
        try:
            res = run_bass_kernel_spmd(nc, in_maps,
                                       core_ids=list(range(N_CORES)),
                                       trace=trace)
            break
        except Exception as e:  # transient device/tunnel hiccups
            last_err = e
    else:
        raise last_err
    out = np.stack([res.results[c]["out"] for c in range(N_CORES)], axis=0)
    return out.astype(np.float32), res


def kernel(x, w):
    out, _ = run(x, w, trace=False)
    return out
